# revision 61
# baseline (speedup 1.0000x reference)
"""Trainium2 Bass kernel for 16-head self-attention (b=2, n=2048, dm=1024, dh=64).

Sharding: each of 8 cores owns (batch g = c//4, sequence block r = c%4).
A core computes K,V for its batch's FULL sequence (replicated across the 4
cores of that batch -- avoids cross-core collectives entirely; the collective
cost model shows ~15us launch + <=110 GB/s, far worse than the 27us of
duplicated projection PE time), attention for all 16 heads restricted to its
512 query rows, and the output projection for those rows.  Per-core outputs
are disjoint [512, 1024] slices of the final [2, 2048, 1024]; the host
concatenates.

Host-side layout trick (kept from the fp32r baseline): the host passes x^T
(dm-major) ROTATED by the core's row offset, so every core's query slice is
columns 0:512 of its own x^T -- the SPMD program is identical across cores.
Attention is permutation-invariant over keys, so the rotation does not change
the result.

v2 redesign, 370us fp32r baseline -> ~286us measured (measured baseline
bottlenecks: PE busy 292us of 360us span at 495ps/row effective vs the
417ps/row hw floor; LdWeights 195ns/matmul only partially hidden; 37us of
PE gaps mostly from the V-read descriptor storm + phase tails; DVE
reciprocal 3.35us/head):

* All matmul operands bf16 (host casts).  Same 1 cycle/row stream rate as
  fp32r, but LdWeights halves (~97ns, hidden behind the 216ns stream), DMA
  bytes halve, and DVE copies get the 16-bit 2x mode.  The head-pair
  zero-padded Q layout (K=128 S matmuls) is KEPT: bf16 K=64 matmuls
  measured ~18us slower end-to-end despite the cost model saying 1
  cycle/row (spad flag A/Bs this).
* K^T and V never round-trip through DRAM: projection PSUM results are
  copied straight into persistent SBUF tiles (KT_sb 4MB, V_sb 8MB).  This
  kills 32MB of DMA traffic and the 2048-descriptor-per-head strided V
  reads that caused the baseline's second-half PE gaps.
* V_sb layout [128 keys, kb, head, 128] where columns 64:128 are ONES:
  the O=attn@V matmul's pad columns compute the softmax denominator
  replicated across PSUM partitions 64:127 for free -- no DRAM-bounce
  broadcast.  The denominator rows are copied down to partitions 0:63
  (the custom-DVE reciprocal_approx_fast mis-addresses partition-shifted
  inputs), inverted (~51 ULP, fine: denominators are sums of 2048
  positive exps), and multiplied into OT.
* exp runs on ACT in 1024-wide instructions (two 512-col key blocks per
  activation) to amortize the ~280ns fixed PSUM-read cost: ACT totals
  ~132us, under the ~249us PE floor.
* Weights are host-packed into per-chain blocks ([ib, 128, a, 128]) so
  both DMA sides are contiguous per partition, and the 32 initial DMA
  issues (~0.6us descriptor-gen EACH, serializing per engine) are split
  across the sync and scalar queues; gpsimd descriptor-gen is 4-8us per
  DMA -- never issue from there.
* The attention phase is ACT-limited (~16.5us/pair vs 13.8us of PE work),
  so independent PE work is interleaved INTO the pairs as filler:
  the vh0 tail into pair 0, V second-half chains into pairs 1-4, and the
  out-projection into pairs 5-7 as closed partial-sum chains (ib 0..4,
  then ib 5..6) accumulated in SBUF (pre_sb, bias folded in).  A filler
  chain must be EMITTED before the matmul that reads its output, else
  Tile sees no dependency edge (reads of never-written regions return
  garbage -- this was a real bug, caught only by NaN-scrubbing SBUF
  between runs; stale SBUF from a previous run otherwise masks it).
  After the last pair only the eight single-matmul ib7 chains + one DVE
  add each remain before the (bf16) output stores.  S->O is
  software-pipelined with a 2-unit lag so O rarely waits on its exp.
"""

import sys

for _p in ("/opt/trn_rl_repo", "/root/.axon_site/_ro/trn_rl_repo"):
    if _p not in sys.path:
        sys.path.append(_p)

import numpy as np

B = 2
N = 2048
DM = 1024
H = 16
DH = 64
INNER = H * DH  # 1024
NCORES = 8
QR = 512  # query rows per core
SCALE = DH ** -0.5

A = DM // 128      # 8 dm blocks
IB = INNER // 128  # 8 inner blocks (head pairs)
KB = N // 128      # 16 key blocks
QB = QR // 128     # 4 query blocks

_cached = {}


def _build(debug=False, prefire=True, vrearrange=True, wexp=True,
           spad=True):
    import contextlib
    import concourse.bacc as bacc
    import concourse.tile as tile
    import concourse.mybir as mybir

    f32 = mybir.dt.float32
    bf16 = mybir.dt.bfloat16
    Exp = mybir.ActivationFunctionType.Exp

    nc = bacc.Bacc("TRN2", target_bir_lowering=False, debug=False,
                   enable_asserts=False)

    # Wq/Wk host-packed as [ib, a, 128dm, 128col] blocks and Wv as
    # [ic, a, 128dm, 512col] so the first projection chain's weights are
    # the FIRST contiguous 256KB off DRAM (Q starts ~2us in, not ~12us)
    xT_d = nc.dram_tensor("xT", [DM, N], bf16, kind="ExternalInput").ap()
    Wq_d = nc.dram_tensor("Wq", [IB, 128, A, 128], bf16,
                          kind="ExternalInput").ap()
    Wk_d = nc.dram_tensor("Wk", [IB, 128, A, 128], bf16,
                          kind="ExternalInput").ap()
    Wv_d = nc.dram_tensor("Wv", [2, 128, A, 512], bf16,
                          kind="ExternalInput").ap()
    Wo_d = nc.dram_tensor("Wo", [INNER, DM], bf16, kind="ExternalInput").ap()
    bo_d = nc.dram_tensor("bo", [DM], f32, kind="ExternalInput").ap()
    out_d = nc.dram_tensor("out", [QR, DM], bf16,
                           kind="ExternalOutput").ap()
    if debug:
        qt_dbg = nc.dram_tensor("qt_dbg", [128, IB, 2, QR], bf16,
                                kind="ExternalOutput").ap()
        kt_dbg = nc.dram_tensor("kt_dbg", [128, IB, N], bf16,
                                kind="ExternalOutput").ap()
        v_dbg = nc.dram_tensor("v_dbg", [128, KB, H, 128], bf16,
                               kind="ExternalOutput").ap()
        ot_dbg = nc.dram_tensor("ot_dbg", [128, IB, QR], bf16,
                                kind="ExternalOutput").ap()
        oraw_dbg = nc.dram_tensor("oraw_dbg", [128, 2, QR], bf16,
                                  kind="ExternalOutput").ap()
        den_dbg = nc.dram_tensor("den_dbg", [64, 2, QR], bf16,
                                 kind="ExternalOutput").ap()
        rcp_dbg = nc.dram_tensor("rcp_dbg", [64, 2, QR], bf16,
                                 kind="ExternalOutput").ap()

    xT_r = xT_d.rearrange("(a p) n -> a p n", p=128)
    Wo_r = Wo_d.rearrange("(ib p) d -> ib p d", p=128)

    with tile.TileContext(nc) as tc, \
         nc.allow_low_precision(reason="bf16 matmul pipeline, validated e2e"), \
         contextlib.ExitStack() as ctx:
        persist = ctx.enter_context(tc.tile_pool(name="persist", bufs=1))
        xT_sb = persist.tile([128, A, N], bf16)        # 32 KB/part
        KT_sb = persist.tile([128, IB, N], bf16)       # 32 KB/part
        V_sb = persist.tile([128, KB, H, 128], bf16)   # 64 KB/part
        if spad:  # zero-padded head-pair Q (fp32r-era layout, K=128 S mm)
            QT_z = persist.tile([128, IB, 2, QR], bf16, name="QT_z")
            QT_sb = None
        else:     # plain Q^T; S matmuls contract K=64 at full bf16 rate
            QT_z = None
            QT_sb = persist.tile([128, IB, QR], bf16, name="QT_sb")
        OT_sb = persist.tile([128, IB, QR], bf16)      # 8 KB/part
        onef = persist.tile([128, 1], f32)
        zerof = persist.tile([128, 1], f32)
        o_raw_sb = (persist.tile([128, 2, QR], bf16, name="o_raw_sb")
                    if debug else None)
        den_keep = (persist.tile([64, 2, QR], bf16, name="den_keep")
                    if debug else None)
        rcp_keep = (persist.tile([64, 2, QR], bf16, name="rcp_keep")
                    if debug else None)

        nc.vector.memset(onef, 1.0)
        nc.vector.memset(zerof, 0.0)
        # one-time fills (DVE: gpsimd broadcast copies are not reliable)
        if spad:
            nc.vector.tensor_copy(
                out=QT_z[:, :, :, :],
                in_=zerof.unsqueeze(1).unsqueeze(1).to_broadcast(
                    [128, IB, 2, QR]))
        nc.vector.tensor_copy(
            out=V_sb[:, :, :, 64:128],
            in_=onef.unsqueeze(1).unsqueeze(1).to_broadcast(
                [128, KB, H, 64]))

        # ---------------- helpers ----------------
        def v_chain(ic, kb, Wv_sb, ps_proj, on_act=False):
            """Project V for key block kb, heads 8ic..8ic+7 (one 8-mm chain)."""
            vp = ps_proj.tile([128, 512], f32, tag="vp", name="vp")
            for a in range(A):
                nc.tensor.matmul(
                    out=vp,
                    lhsT=xT_sb[:, a, kb * 128:(kb + 1) * 128],
                    rhs=Wv_sb[:, ic, a, :],
                    start=(a == 0), stop=(a == A - 1))
            if on_act:  # ACT is idle before the first exp; unload DVE
                nc.scalar.activation(
                    out=V_sb[:, kb, ic * 8:(ic + 1) * 8, 0:64],
                    in_=vp.rearrange("p (h c) -> p h c", h=8),
                    func=mybir.ActivationFunctionType.Copy)
            elif vrearrange:
                nc.vector.tensor_copy(
                    out=V_sb[:, kb, ic * 8:(ic + 1) * 8, 0:64],
                    in_=vp.rearrange("p (h c) -> p h c", h=8))
            else:
                for hq in range(8):
                    nc.vector.tensor_copy(
                        out=V_sb[:, kb, ic * 8 + hq, 0:64],
                        in_=vp[:, hq * 64:(hq + 1) * 64])

        def attn_pair(hp, sp_pool, op_pool, es_pool, rc_pool, filler):
            """Attention for head pair hp over this core's 512 queries.

            16 units (2 heads x 8 double-key-blocks); O lags S by 2 units so
            exp (ACT) is never on the PE critical path; `filler` yields
            callables that emit independent PE work into designated slots.
            """
            units = [(hh, j) for hh in range(2) for j in range(KB // 2)]
            ops = {}
            pending = []  # (hh, j, es_tile)

            def s_ops(hh, kb):
                if spad:
                    return (KT_sb[:, hp, kb * 128:(kb + 1) * 128],
                            QT_z[:, hp, hh, :])
                p0 = hh * 64
                return (KT_sb[p0:p0 + 64, hp, kb * 128:(kb + 1) * 128],
                        QT_sb[p0:p0 + 64, hp, :])

            def emit_S(hh, j):
                es = es_pool.tile([128, 2, 512], bf16, tag="es", name="es")
                if wexp:
                    sp = sp_pool.tile([128, 2, 512], f32, tag="sp", name="sp")
                    for u in range(2):
                        lhsT, rhs = s_ops(hh, 2 * j + u)
                        nc.tensor.matmul(out=sp[:, u, :], lhsT=lhsT, rhs=rhs,
                                         start=True, stop=True)
                    nc.scalar.activation(out=es, in_=sp, func=Exp,
                                         scale=SCALE)
                else:
                    for u in range(2):
                        sp = sp_pool.tile([128, 2, 512], f32, tag="sp",
                                          name="sp")
                        lhsT, rhs = s_ops(hh, 2 * j + u)
                        nc.tensor.matmul(out=sp[:, 0, :], lhsT=lhsT, rhs=rhs,
                                         start=True, stop=True)
                        nc.scalar.activation(out=es[:, u, :], in_=sp[:, 0, :],
                                             func=Exp, scale=SCALE)
                pending.append((hh, j, es))

            def emit_O():
                hh, j, es = pending.pop(0)
                h = hp * 2 + hh
                if j == 0:
                    ops[hh] = op_pool.tile([128, QR], f32, tag="op",
                                           name="op")
                for u in range(2):
                    kb = 2 * j + u
                    nc.tensor.matmul(
                        out=ops[hh],
                        lhsT=V_sb[:, kb, h, :],
                        rhs=es[:, u, :],
                        start=(kb == 0), stop=(kb == KB - 1))
                if j == KB // 2 - 1:
                    if o_raw_sb is not None and h < 2:
                        nc.vector.tensor_copy(out=o_raw_sb[:, h, :],
                                              in_=ops[hh])
                    # rows 64:128 of op are the softmax denominator
                    # (replicated by V_sb's ones columns).  Copy them down
                    # to partitions 0:63 first: the custom-DVE reciprocal
                    # mis-reads partition-shifted inputs (it reuses the
                    # output's base partition), so in/out must be aligned.
                    den = rc_pool.tile([64, QR], f32, tag="den", name="den")
                    nc.vector.tensor_copy(out=den, in_=ops[hh][64:128, :])
                    rcp = rc_pool.tile([64, QR], f32, tag="rcp", name="rcp")
                    nc.vector.reciprocal_approx_fast(out=rcp, in_=den)
                    if den_keep is not None and h < 2:
                        nc.vector.tensor_copy(out=den_keep[:, h, :], in_=den)
                        nc.vector.tensor_copy(out=rcp_keep[:, h, :], in_=rcp)
                    nc.vector.tensor_mul(
                        OT_sb[hh * 64:(hh + 1) * 64, hp, :],
                        ops[hh][0:64, :], rcp)

            for t, (hh, j) in enumerate(units):
                emit_S(hh, j)
                if t % 4 == 3 and filler:
                    filler.pop(0)()
                if t >= 2:
                    emit_O()
            while pending:  # interleave leftover filler into the drain:
                if filler:  # the last O units otherwise stall on the final
                    filler.pop(0)()  # exps (ACT runs ~2.7us behind by now)
                emit_O()
            while filler:
                filler.pop(0)()

        # ---------------- load + projections ----------------
        with tc.tile_pool(name="p_wqk", bufs=1) as p_wqk, \
             tc.tile_pool(name="ps_qk", bufs=2, space="PSUM") as ps_qk:
            Wq_sb = p_wqk.tile([128, IB, A, 128], bf16)
            Wk_sb = p_wqk.tile([128, IB, A, 128], bf16)
            # issue the 32 initial DMAs from 4 engines in parallel --
            # descriptor generation is ~0.6us per dma_start and serializes
            # per engine, so a single queue costs ~19us before K can start
            nc.sync.dma_start(out=Wq_sb[:, 0, :, :], in_=Wq_d[0])
            for a in range(A):  # Q only needs x columns 0:QR; alternate
                eng = nc.scalar if a % 2 == 0 else nc.sync  # queues so all
                eng.dma_start(out=xT_sb[:, a, 0:QR],  # 8 land ~2us earlier
                              in_=xT_r[a, :, 0:QR])
            for ib in range(1, IB):
                nc.sync.dma_start(out=Wq_sb[:, ib, :, :], in_=Wq_d[ib])
            for ib in range(IB):
                nc.sync.dma_start(out=Wk_sb[:, ib, :, :], in_=Wk_d[ib])
            for a in range(A):
                nc.scalar.dma_start(out=xT_sb[:, a, QR:N],
                                    in_=xT_r[a, :, QR:N])

            for ib in range(IB):
                qp = ps_qk.tile([128, QR], f32, tag="proj")
                for a in range(A):
                    nc.tensor.matmul(
                        out=qp,
                        lhsT=Wq_sb[:, ib, a, :],
                        rhs=xT_sb[:, a, 0:QR],
                        start=(a == 0), stop=(a == A - 1))
                if spad:
                    nc.vector.tensor_copy(out=QT_z[0:64, ib, 0, :],
                                          in_=qp[0:64, :])
                    nc.vector.tensor_copy(out=QT_z[64:128, ib, 1, :],
                                          in_=qp[64:128, :])
                else:
                    nc.vector.tensor_copy(out=QT_sb[:, ib, :], in_=qp)
            def k_chain(ib, kc):
                kp = ps_qk.tile([128, 512], f32, tag="proj")
                for a in range(A):
                    nc.tensor.matmul(
                        out=kp,
                        lhsT=Wk_sb[:, ib, a, :],
                        rhs=xT_sb[:, a, kc * 512:(kc + 1) * 512],
                        start=(a == 0), stop=(a == A - 1))
                nc.vector.tensor_copy(
                    out=KT_sb[:, ib, kc * 512:(kc + 1) * 512], in_=kp)

            for kc in range(N // 512):  # kc-outer: kc=0 needs only the
                for ib in range(IB):    # already-loaded query columns of x
                    k_chain(ib, kc)

        # ---------------- V + attention, interleaved ----------------
        with tc.tile_pool(name="ps_sp", bufs=2, space="PSUM") as sp_pool, \
             tc.tile_pool(name="ps_op", bufs=2, space="PSUM") as op_pool, \
             tc.tile_pool(name="p_es", bufs=6) as es_pool, \
             tc.tile_pool(name="p_rc", bufs=2) as rc_pool, \
             tc.tile_pool(name="p_wo1", bufs=1) as p_wo1:
            Wo_sb = p_wo1.tile([128, IB, DM], bf16)
            for ib in range(IB):  # early: pair-5 prefire chains need it
                nc.sync.dma_start(out=Wo_sb[:, ib, :], in_=Wo_r[ib])

            with tc.tile_pool(name="p_wv", bufs=1) as p_wv, \
                 tc.tile_pool(name="ps_v", bufs=2, space="PSUM") as ps_v:
                Wv_sb = p_wv.tile([128, 2, A, 512], bf16)
                for ic in range(2):
                    nc.sync.dma_start(out=Wv_sb[:, ic, :, :], in_=Wv_d[ic])
                for kb in range(KB - 4):  # V heads 0..7, key blocks 0..11
                    v_chain(0, kb, Wv_sb, ps_v)

                def vc(ic, kbs):
                    def f():
                        for kb in kbs:
                            v_chain(ic, kb, Wv_sb, ps_v)
                    return f

                def vc_split(ic, kb):
                    """One V chain as two 4-mm callables (the second lands
                    in the pair's drain, PE filler for the final exps)."""
                    box = {}

                    def h1():
                        vp = ps_v.tile([128, 512], f32, tag="vp", name="vp")
                        box["vp"] = vp
                        for a in range(4):
                            nc.tensor.matmul(
                                out=vp,
                                lhsT=xT_sb[:, a, kb * 128:(kb + 1) * 128],
                                rhs=Wv_sb[:, ic, a, :],
                                start=(a == 0), stop=False)

                    def h2():
                        vp = box["vp"]
                        for a in range(4, A):
                            nc.tensor.matmul(
                                out=vp,
                                lhsT=xT_sb[:, a, kb * 128:(kb + 1) * 128],
                                rhs=Wv_sb[:, ic, a, :],
                                start=False, stop=(a == A - 1))
                        nc.vector.tensor_copy(
                            out=V_sb[:, kb, ic * 8:(ic + 1) * 8, 0:64],
                            in_=vp.rearrange("p (h c) -> p h c", h=8))

                    return h1, h2

                # filler emission deadlines: pair p's O matmuls read V for
                # ALL key blocks of its heads, so vh0's tail chains must be
                # emitted by pair 0's slots t=3/t=7 (read at units 8/9) and
                # every vh1 chain before pair 4.  Fillers also keep the PE
                # busy while ACT catches up on exps (17.9us vs 13.65us/pair).
                # pairs 0-3 read vh0 (heads 0-7), pairs 4-7 read vh1:
                # each chain must be EMITTED before the O matmul that reads
                # it (fillers run before emit_O of the same t), hence vh0's
                # tail sits in pair 0's slots and vh1 finishes by pair 4's
                # t=7 slot.  ~6.8us filler per pair covers the ~4.3us/pair
                # ACT deficit.
                attn_pair(0, sp_pool, op_pool, es_pool, rc_pool,
                          [vc(0, [12, 13]), vc(0, [14, 15])])
                attn_pair(1, sp_pool, op_pool, es_pool, rc_pool,
                          [vc(1, [0, 1]), vc(1, [2, 3])])
                attn_pair(2, sp_pool, op_pool, es_pool, rc_pool,
                          [vc(1, [4, 5]), vc(1, [6, 7])])
                attn_pair(3, sp_pool, op_pool, es_pool, rc_pool,
                          [vc(1, [8, 9]), vc(1, [10, 11])])
                attn_pair(4, sp_pool, op_pool, es_pool, rc_pool,
                          [vc(1, [12, 13]), vc(1, [14, 15])])

            with tc.tile_pool(name="p_wo", bufs=1) as p_wo, \
                 tc.tile_pool(name="ps_oc", bufs=2, space="PSUM") as ps_oc, \
                 tc.tile_pool(name="p_ob", bufs=4) as p_ob:
                bo_sb = p_wo.tile([128, DM], f32)
                pre_sb = p_wo.tile([128, 2, QB, 512], bf16)
                nc.gpsimd.dma_start(
                    out=bo_sb, in_=bo_d.unsqueeze(0).to_broadcast([128, DM]))

                # out-projection as prefired partial sums: ib7 (= head pair
                # 7, finished last) would otherwise gate all 64 matmuls of
                # the projection, a ~12us serial tail.  Instead pre_sb
                # accumulates ib0..4 (chains emitted as pair-5/6 filler:
                # those OT blocks are complete) and ib5..6 (pair-7 filler),
                # each chain closed and drained to SBUF so two PSUM banks
                # serve all 8 chains.  After pair 7 only the single-matmul
                # ib7 chains + one DVE add per output remain.
                def chainA(dc, qb):  # ib 0..4, + bias folded in
                    t = ps_oc.tile([128, 512], f32, tag="oc", name="oc")
                    for ib in range(5):
                        nc.tensor.matmul(
                            out=t,
                            lhsT=OT_sb[:, ib, qb * 128:(qb + 1) * 128],
                            rhs=Wo_sb[:, ib, dc * 512:(dc + 1) * 512],
                            start=(ib == 0), stop=(ib == 4))
                    nc.vector.tensor_add(
                        pre_sb[:, dc, qb, :], t,
                        bo_sb[:, dc * 512:(dc + 1) * 512])

                def chainB(dc, qb):  # ib 5..6, accumulated into pre_sb
                    t = ps_oc.tile([128, 512], f32, tag="oc", name="oc")
                    for ib in range(5, 7):
                        nc.tensor.matmul(
                            out=t,
                            lhsT=OT_sb[:, ib, qb * 128:(qb + 1) * 128],
                            rhs=Wo_sb[:, ib, dc * 512:(dc + 1) * 512],
                            start=(ib == 5), stop=(ib == 6))
                    nc.vector.tensor_add(
                        pre_sb[:, dc, qb, :], t, pre_sb[:, dc, qb, :])

                def many(fn, args):
                    def f():
                        for a in args:
                            fn(*a)
                    return f

                QD = [(dc, qb) for dc in range(2) for qb in range(QB)]
                if prefire:
                    attn_pair(5, sp_pool, op_pool, es_pool, rc_pool,
                              [many(chainA, QD[0:2]), many(chainA, QD[2:4])])
                    attn_pair(6, sp_pool, op_pool, es_pool, rc_pool,
                              [many(chainA, QD[4:6]), many(chainA, QD[6:8])])
                    attn_pair(7, sp_pool, op_pool, es_pool, rc_pool,
                              [many(chainB, QD[0:3]), many(chainB, QD[3:5]),
                               many(chainB, QD[5:6]), many(chainB, QD[6:7]),
                               many(chainB, QD[7:8])])
                else:
                    attn_pair(5, sp_pool, op_pool, es_pool, rc_pool, [])
                    attn_pair(6, sp_pool, op_pool, es_pool, rc_pool, [])
                    attn_pair(7, sp_pool, op_pool, es_pool, rc_pool, [])
                    for dc, qb in QD:
                        chainA(dc, qb)
                    for dc, qb in QD:
                        chainB(dc, qb)

                for dc, qb in QD:  # only ib7 remains after the last pair
                    t = ps_oc.tile([128, 512], f32, tag="oc", name="oc")
                    nc.tensor.matmul(
                        out=t,
                        lhsT=OT_sb[:, IB - 1, qb * 128:(qb + 1) * 128],
                        rhs=Wo_sb[:, IB - 1, dc * 512:(dc + 1) * 512],
                        start=True, stop=True)
                    ob = p_ob.tile([128, 512], bf16, tag="ob", name="ob")
                    nc.vector.tensor_add(ob, t, pre_sb[:, dc, qb, :])
                    nc.sync.dma_start(
                        out=out_d[qb * 128:(qb + 1) * 128,
                                  dc * 512:(dc + 1) * 512],
                        in_=ob)

        if debug:
            nc.sync.dma_start(out=qt_dbg, in_=QT_z[:, :, :, :])
            nc.sync.dma_start(out=kt_dbg, in_=KT_sb[:, :, :])
            nc.sync.dma_start(out=v_dbg, in_=V_sb[:, :, :, :])
            nc.sync.dma_start(out=ot_dbg, in_=OT_sb[:, :, :])
            nc.sync.dma_start(out=oraw_dbg, in_=o_raw_sb[:, :, :])
            nc.sync.dma_start(out=den_dbg, in_=den_keep[:, :, :])
            nc.sync.dma_start(out=rcp_dbg, in_=rcp_keep[:, :, :])

    nc.compile()
    return nc


def _get_nc():
    if "nc" not in _cached:
        _cached["nc"] = _build()
    return _cached["nc"]


def kernel(queries, Wq, Wkv, Wo, bo, _trace=False):
    from concourse.bass_utils import run_bass_kernel_spmd
    import ml_dtypes

    bf16 = ml_dtypes.bfloat16
    queries = np.asarray(queries, dtype=np.float32)
    Wkv = np.asarray(Wkv, dtype=np.float32)

    def pack_blocks(W, cols):  # [DM, INNER] -> [INNER//cols, 128, A, cols]
        # partition-major inside each block so every SBUF partition's DMA
        # read is one contiguous A*cols*2-byte run (no descriptor storm)
        return np.ascontiguousarray(
            W.reshape(A, 128, INNER // cols, cols).transpose(2, 1, 0, 3)
        ).astype(bf16)

    Wq_c = pack_blocks(np.asarray(Wq, dtype=np.float32), 128)
    Wk_c = pack_blocks(Wkv[:, :INNER], 128)
    Wv_c = pack_blocks(Wkv[:, INNER:], 512)
    Wo_c = np.asarray(Wo, dtype=np.float32).astype(bf16)
    bo = np.asarray(bo, dtype=np.float32)

    nc = _get_nc()

    in_maps = []
    for c in range(NCORES):
        g, r = c // 4, c % 4
        xT = np.ascontiguousarray(queries[g].T)          # [DM, N]
        xT = np.ascontiguousarray(np.roll(xT, -r * QR, axis=1)).astype(bf16)
        in_maps.append({"xT": xT, "Wq": Wq_c, "Wk": Wk_c, "Wv": Wv_c,
                        "Wo": Wo_c, "bo": bo})

    res = run_bass_kernel_spmd(nc, in_maps, list(range(NCORES)),
                               trace=_trace)
    out = np.empty((B, N, DM), dtype=np.float32)
    for c in range(NCORES):
        g, r = c // 4, c % 4
        out[g, r * QR:(r + 1) * QR, :] = np.asarray(
            res.results[c]["out"], dtype=np.float32)
    if _trace:
        return out, res
    return out


if __name__ == "__main__":
    rng = np.random.default_rng(0)
    q = rng.standard_normal((B, N, DM), dtype=np.float32)
    s = 0.02
    inputs = dict(
        queries=q,
        Wq=(rng.standard_normal((DM, INNER), dtype=np.float32) * s),
        Wkv=(rng.standard_normal((DM, 2 * INNER), dtype=np.float32) * s),
        Wo=(rng.standard_normal((INNER, DM), dtype=np.float32) * s),
        bo=(rng.standard_normal((DM,), dtype=np.float32) * s),
    )
    out = kernel(**inputs)
    print("kernel ran, out shape", out.shape)
